# revision 15
# baseline (speedup 1.0000x reference)
"""Expert-parallel MoE MLP kernel for Trainium2 (8 NeuronCores).

Problem: x[B=2,S=1024,H=1024] f32, expert_indices[B,S] int, 16 experts,
gate/up_proj[E,H,I], down_proj[E,I,H] (H=I=1024):
    out[n] = silu(x_n @ Wg[e_n]) * (x_n @ Wu[e_n]) @ Wd[e_n].T

Sharding: expert parallelism — core c owns experts {2c, 2c+1}. The host
groups tokens by expert (the "all-to-all dispatch" runs on host since the
kernel contract is full-input -> full-output), pads each expert's token
block to a multiple of 16, and each core runs dense per-expert GEMMs.

Device layout (per core, per expert e) keeps features on partitions so no
on-chip transposes are needed:
    xt    = X_e^T                [H=1024, pio]
    Gt[i,n] = sum_h Wg[h,i]*xt[h,n];  inter = silu(Gt)*Ut
    Out^T[j,n] = sum_k WdT[k,j]*inter[k,n]   (WdT = Wd.T, host-transposed)

Everything DMA'd is bfloat16 (weights, xt, out) — the kernel is HBM-bound
(12 MB/core of weights is mandatory traffic; per-core DMA line rate is
16 engines x 22.5 B/ns = 360 GB/s), so halving bytes vs f32 halves runtime.
bf16 matmul runs at 1 cycle/row at ANY moving size (unlike f32r which
needs >=256), so the moving dim is the real padded token count pio and the
pad region needs no on-chip memset (host zero-fills it). Rel err ~2e-3.

Perf structure:
  - all weight+xt DMAs are issued up-front on the Sync HWDGE queue in
    stream order (xt0, Wg0, xt1, Wu0, Wd0, Wg1, Wu1, Wd1) and the weight
    pool holds every tile, so the queue never stalls on buffer recycling
    and the 16 DMA engines run at line rate end to end
  - weights are host-packed partition-major so each 0.5 MB quarter moves
    4 KB contiguous per-partition runs (>=512B keeps full descriptor rate)
  - per-expert compute (gate -> silu -> up -> mul -> down) streams behind
    the weight DMAs with ~2x slack on the Tensor engine
  - outputs are staged in SBUF as bf16 and shipped on the Scalar (ACT)
    HWDGE queue so the output triggers never block the weight FIFO
"""

import math

import numpy as np

E = 16
H = 1024
HT = 8          # H / 128 partition tiles
N_CORES = 8
EPC = E // N_CORES  # experts per core
NS = 4          # weight DMA quarters per projection (tile = [128, 2, H])
HH = HT // NS   # h-blocks per weight tile

_NC_CACHE = {}


def _build_nc(pio: int):
    """One SPMD program: EPC experts, pio token-slots per expert (mult of 16)."""
    import concourse.tile as tile
    from concourse import bacc, mybir
    from concourse.bass import ts

    f32 = mybir.dt.float32
    bf16 = mybir.dt.bfloat16

    nc = bacc.Bacc("TRN2", target_bir_lowering=False, debug=False,
                   num_devices=N_CORES)
    # weights packed partition-major: w[e, proj, p, h, :] = Wproj[e][h*128+p, :]
    w = nc.dram_tensor("w", [EPC, 3, 128, HT, H], bf16, kind="ExternalInput")
    xt = nc.dram_tensor("xt", [128, EPC, HT, pio], bf16, kind="ExternalInput")
    out = nc.dram_tensor("out", [EPC, 128, HT, pio], bf16, kind="ExternalOutput")

    LAST = EPC - 1
    with tile.TileContext(nc) as tc:
        with (
            tc.tile_pool(name="wph", bufs=EPC * 2 * 2) as wph,
            tc.tile_pool(name="wpq", bufs=EPC * NS) as wpq,
            tc.tile_pool(name="xp", bufs=1) as xp,
            tc.tile_pool(name="gp", bufs=2) as gp,
            tc.tile_pool(name="ip", bufs=2) as ip,
            tc.tile_pool(name="op", bufs=2) as op,
            tc.tile_pool(name="ps", bufs=8, space="PSUM") as ps,
        ):
            x_sb = xp.tile([128, EPC, HT, pio], bf16)
            halves = {}

            def w_tile(e, proj, hf, hh):
                """DMA h-blocks [hf*hh, (hf+1)*hh) of projection (e, proj)."""
                pool = wpq if hh == HH else wph
                t = pool.tile([128, hh, H], bf16, tag="w", name=f"wh{e}{proj}{hf}")
                nc.sync.dma_start(t[:], w[e, proj, :, ts(hf, hh), :])
                for i in range(hh):
                    halves[e, proj, hf * hh + i] = (t, i)

            def w_proj(e, proj):
                # Tile sizes balance two constraints: the HWDGE ring allows
                # only 8 outstanding DMAs (smaller tiles starve the stream
                # when trigger/slot cadence exceeds transfer time; fewer DMAs
                # ahead of the tail let the final quarters' triggers fire
                # early), while the tail wants the last tiles small (less
                # post-stream matmul work). Everything streams as 1 MB
                # halves except the last expert's down projection (0.5 MB
                # quarters).
                if (e, proj, 0) in halves:
                    return
                if e == LAST and proj == 2:
                    for hf in range(NS):
                        w_tile(e, proj, hf, HH)
                else:
                    for hf in range(2):
                        w_tile(e, proj, hf, HT // 2)

            # Input stream order: both experts' gate/up weights first, down
            # weights last. Everything except the final down quarters (and
            # their 16-matmul burst -> casts -> out DMA) is then off the
            # post-stream critical path: silu/mul chains for both experts
            # complete deep inside the DMA window.
            nc.sync.dma_start(x_sb[:], xt[:])
            for e in range(EPC):
                w_proj(e, 0)
                w_proj(e, 1)
            for e in range(EPC):
                w_proj(e, 2)

            def wsl(e, proj, h, col):
                t, i = halves[e, proj, h]
                return t[:, i, col]

            # phase 1 per expert: gate -> silu, up -> inter
            i_sbs = []
            for e in range(EPC):
                g_sb = gp.tile([128, HT, pio], f32)       # silu(Gt)
                i_sb = ip.tile([128, HT, pio], bf16)      # inter = silu(Gt)*Ut
                i_sbs.append(i_sb)
                # gate: h-outer accumulation, one PSUM tile per i-block
                g_ps = [ps.tile([128, 256], f32, tag="ps", name=f"gps{i_}")
                        for i_ in range(HT)]
                for h in range(HT):
                    for i in range(HT):
                        nc.tensor.matmul(
                            g_ps[i][:, 0:pio], wsl(e, 0, h, ts(i, 128)),
                            x_sb[:, e, h, :],
                            start=(h == 0), stop=(h == HT - 1))
                for i in range(HT):
                    nc.scalar.activation(
                        g_sb[:, i, :], g_ps[i][:, 0:pio],
                        mybir.ActivationFunctionType.Silu)
                # up
                u_ps = [ps.tile([128, 256], f32, tag="ps", name=f"ups{i_}")
                        for i_ in range(HT)]
                for h in range(HT):
                    for i in range(HT):
                        nc.tensor.matmul(
                            u_ps[i][:, 0:pio], wsl(e, 1, h, ts(i, 128)),
                            x_sb[:, e, h, :],
                            start=(h == 0), stop=(h == HT - 1))
                for i in range(HT):
                    nc.vector.tensor_mul(
                        i_sb[:, i, :], g_sb[:, i, :], u_ps[i][:, 0:pio])

            # phase 2 per expert: down -> out
            for e in range(EPC):
                i_sb = i_sbs[e]
                o_sb = op.tile([128, HT, pio], bf16, tag="o")
                o_ps = [ps.tile([128, 256], f32, tag="ps", name=f"ops{i_}")
                        for i_ in range(HT)]
                for k in range(HT):
                    for j in range(HT):
                        nc.tensor.matmul(
                            o_ps[j][:, 0:pio], wsl(e, 2, k, ts(j, 128)),
                            i_sb[:, k, :],
                            start=(k == 0), stop=(k == HT - 1))
                # PSUM->SBUF copies split across Vector and Scalar so the
                # tail isn't serialized on one engine
                for j in range(HT // 2):
                    nc.vector.tensor_copy(o_sb[:, j, :], o_ps[j][:, 0:pio])
                for j in range(HT // 2, HT):
                    nc.scalar.copy(o_sb[:, j, :], o_ps[j][:, 0:pio])
                # Outputs ride the Sync queue (its descriptors enqueue after
                # every weight descriptor, so they never steal mid-stream
                # bandwidth); the last expert's second half goes on the
                # Scalar queue so both halves ship in parallel at the tail.
                nc.sync.dma_start(out[e, :, 0:HT // 2, :], o_sb[:, 0:HT // 2, :])
                (nc.scalar if e == LAST else nc.sync).dma_start(
                    out[e, :, HT // 2:HT, :], o_sb[:, HT // 2:HT, :])
    nc.compile()
    return nc


def _get_nc(pio: int):
    if pio not in _NC_CACHE:
        _NC_CACHE[pio] = _build_nc(pio)
    return _NC_CACHE[pio]


_ROUND_CAP = 256          # max tokens/expert per device round


def _kernel_once(x, expert_indices, gate_proj, up_proj, down_proj):
    import ml_dtypes
    from concourse.bass_utils import run_bass_kernel_spmd

    bf16 = np.dtype(ml_dtypes.bfloat16)
    x = np.ascontiguousarray(x, dtype=np.float32)
    b, s, h = x.shape
    assert (h, gate_proj.shape) == (H, (E, H, H)), (x.shape, gate_proj.shape)

    n = b * s
    xf = x.reshape(n, h)
    idx = np.asarray(expert_indices).reshape(n).astype(np.int64)

    order = np.argsort(idx, kind="stable")       # token ids grouped by expert
    counts = np.bincount(idx, minlength=E)
    starts = np.zeros(E + 1, dtype=np.int64)
    np.cumsum(counts, out=starts[1:])
    maxc = int(counts.max())
    assert maxc <= _ROUND_CAP
    pio = max(16, 16 * math.ceil(maxc / 16))

    # per-core inputs; weights packed partition-major [EPC,3,128,HT,H]
    wr = np.stack(
        [np.asarray(gate_proj), np.asarray(up_proj),
         np.asarray(down_proj).transpose(0, 2, 1)], axis=1
    ).astype(bf16).reshape(N_CORES, EPC, 3, HT, 128, H).transpose(0, 1, 2, 4, 3, 5)
    in_maps = []
    tok_ids = []
    for c in range(N_CORES):
        xt_c = np.zeros((EPC, H, pio), dtype=np.float32)
        toks = []
        for le in range(EPC):
            e = c * EPC + le
            te = order[starts[e]:starts[e + 1]]
            toks.append(te)
            xt_c[le, :, :len(te)] = xf[te].T
        tok_ids.append(toks)
        in_maps.append({
            "w": np.ascontiguousarray(wr[c]),
            "xt": xt_c.astype(bf16).reshape(EPC, HT, 128, pio)
                  .transpose(2, 0, 1, 3).copy(),
        })

    nc = _get_nc(pio)
    res = run_bass_kernel_spmd(nc, in_maps, core_ids=list(range(N_CORES)))

    out = np.empty((n, h), dtype=np.float32)
    for c in range(N_CORES):
        o = res.results[c]["out"].astype(np.float32)   # [EPC, 128, HT, pio]
        for le in range(EPC):
            te = tok_ids[c][le]
            oe = o[le].transpose(1, 0, 2).reshape(h, pio)   # [H, pio]
            out[te] = oe[:, :len(te)].T
    return out.reshape(b, s, h)


def kernel(x, expert_indices, gate_proj, up_proj, down_proj):
    """Full-input -> full-output entry point.

    Tokens-per-expert above _ROUND_CAP (pathological skew; PSUM bound)
    are handled by running the device kernel in multiple rounds over
    disjoint token slices — outputs are per-token independent."""
    idx = np.asarray(expert_indices)
    counts = np.bincount(idx.reshape(-1).astype(np.int64), minlength=E)
    if counts.max() <= _ROUND_CAP:
        return _kernel_once(x, expert_indices, gate_proj, up_proj, down_proj)

    b, s, h = x.shape
    n = b * s
    xf = np.ascontiguousarray(x, dtype=np.float32).reshape(n, h)
    idxf = idx.reshape(n).astype(np.int64)
    order = np.argsort(idxf, kind="stable")
    starts = np.zeros(E + 1, dtype=np.int64)
    np.cumsum(np.bincount(idxf, minlength=E), out=starts[1:])
    out = np.empty((n, h), dtype=np.float32)
    rounds = math.ceil(counts.max() / _ROUND_CAP)
    for r in range(rounds):
        sel = np.concatenate([
            order[starts[e] + r * _ROUND_CAP:
                  min(starts[e] + (r + 1) * _ROUND_CAP, starts[e + 1])]
            for e in range(E)])
        if not len(sel):
            continue
        xr = xf[sel].reshape(1, len(sel), h)
        ir = idxf[sel].reshape(1, len(sel))
        out[sel] = _kernel_once(
            xr, ir, gate_proj, up_proj, down_proj).reshape(len(sel), h)
    return out.reshape(b, s, h)
